# revision 40
# baseline (speedup 1.0000x reference)
"""Single-head attention layer (B=4, S=2048, D=H=1024) on 8 TRN2 NeuronCores.

Sharding: core c -> batch c//2, query-half c%2 (1024 query rows per core).

The Q and K projections are FUSED into one on-device pass via the host-
precomputed matrix M = Wq @ Wk^T:   scores = x M x^T + (x Wk bq)_k
(bk cancels in the softmax; bq's surviving term is per-k and enters as the
exp's per-partition bias). This removes the entire Q-projection phase.
t^T = (M x^T) is computed LOCALLY IN FULL on every core (fp8 DoubleRow
over the whole 2048-column sequence) instead of being exchanged: the CC
stream needs ~40us of init barrier before the first collective completes,
which would starve the scores phase. V still goes through 2-core
AllGathers (they have ~40us of schedule slack before phase C).

Matmul precision: V projection and attn@V run in bf16 with fp32 PSUM
accumulation. The t pass and the scores matmul run in fp8e4 DoubleRow
(two planes per instruction). M is prescaled by 32 before e4m3 encoding -
its entries (~1e-2) would otherwise land in e4m3's subnormal range and
quantize catastrophically; the exp's scale divides the 32 back out.
Numpy-sim rel err 1.44e-2 (sim matched HW to 3 digits on the previous
scheme), within the 2e-2 gate. Q-proj-in-fp8 (2.03e-2) and V-in-fp8
(2.93e-2) variants were simulated and rejected.

A short burst of warmup matmuls on a zeroed SBUF tile runs while the input
DMAs stream in, so the PE HAM clock-gate is already warm when the first
real matmul issues.

  t8[d,k]    = fp8(matmul_DR(lhsT=mt8[d'2,d], rhs=x8f[d'2,k]))  k: all 2048
  Vh[s,h]    = matmul(lhsT=xq[d,s], rhs=wv[d,h]) + bv    (own half; AG pairs)
  ET[k,q]    = exp(matmul_DR(lhsT=t8[d2,k], rhs=xq8[d2,q])/(32 sqrt(H)) + u)
  O[q,h]     = matmul(lhsT=ET[k,q], rhs=V[k,h])   (accumulate over k)
  den[q,1]   = matmul(lhsT=ET[k,q], rhs=ones[k,1])
  out        = O * (1/den)
"""

import os

import numpy as np
import ml_dtypes

B, S, D, H = 4, 2048, 1024, 1024
NCORES = 8
PT = 128            # partition tile
CH = 512            # psum free-dim chunk (fp32 bank limit)
QH = S // 2         # query rows per core
ND = D // PT        # 8 d-tiles
NHT = H // PT       # 8 h-tiles
NKT = S // PT       # 16 k-tiles (full sequence)
NST = QH // PT      # 8 own-half s-tiles
NQT = QH // PT      # 8 q-tiles per core
SCALE = 1.0 / float(np.sqrt(H))
MSC = 32.0          # fused-weight prescale keeping e4m3 out of subnormals
N_WARM = 16         # warmup matmuls to trip the HAM clock-gate early

BF16 = ml_dtypes.bfloat16

_NC = None


def _build():
    import concourse.bacc as bacc
    import concourse.mybir as mybir
    from concourse.tile import TileContext

    dt = mybir.dt
    AF = mybir.ActivationFunctionType
    DR = mybir.MatmulPerfMode.DoubleRow
    GROUPS = [[0, 1], [2, 3], [4, 5], [6, 7]]

    nc = bacc.Bacc(None, target_bir_lowering=False, num_devices=NCORES,
                   num_swdge_queues=4)

    # Inputs are pre-tiled on the host into the exact SBUF layout
    # [PT, ND*X] (partition-major), so every load is one DMA with fully
    # contiguous per-partition descriptors (16KB packets, not 2KB rows).
    xq = nc.declare_dram_parameter("xq", [PT, ND * QH], dt.bfloat16,
                                   isOutput=False)
    x8f = nc.declare_dram_parameter("x8f", [PT, ND * S], dt.float8e4,
                                    isOutput=False)
    xq8 = nc.declare_dram_parameter("xq8", [PT, ND * QH], dt.float8e4,
                                    isOutput=False)
    mt8 = nc.declare_dram_parameter("mt8", [PT, ND * H], dt.float8e4,
                                    isOutput=False)
    wv = nc.declare_dram_parameter("wv", [PT, ND * H], dt.bfloat16,
                                   isOutput=False)
    ut = nc.declare_dram_parameter("ut", [PT, NKT], dt.float32,
                                   isOutput=False)
    bvb = nc.declare_dram_parameter("bvb", [PT, H], dt.bfloat16, isOutput=False)
    y = nc.declare_dram_parameter("y", [QH, H], dt.bfloat16, isOutput=True)

    with TileContext(nc) as tc:
        with (
            tc.tile_pool(name="px", bufs=ND) as px,        # ET tiles
            tc.tile_pool(name="pin", bufs=1) as pin,       # big input tiles
            tc.tile_pool(name="pkt", bufs=1) as pkt,
            tc.tile_pool(name="pv", bufs=1) as pv,         # gathered V blocks
            tc.tile_pool(name="pmisc", bufs=1) as pmisc,
            tc.tile_pool(name="pstage", bufs=4) as pstage,
            tc.tile_pool(name="prd", bufs=2) as prd,
            tc.tile_pool(name="pdram", bufs=1, space="DRAM") as pdram,
            tc.tile_pool(name="psum", bufs=8, space="PSUM") as pp,
        ):
            # ---- PE warmup: full-width matmuls on a zeroed SBUF tile while
            # the input DMAs stream in; trips the HAM clock-gate to full
            # clock before the first real matmul. WAW on the psum tile keeps
            # them back-to-back on the PE queue. ----
            warm = pmisc.tile([PT, CH], dt.bfloat16, tag="warm")
            nc.vector.memset(warm[:], 0.0)
            wps = pp.tile([PT, CH], dt.float32, tag="big", name="psb")
            for _ in range(N_WARM):
                nc.tensor.matmul(wps[:], warm[:, 0:PT], warm[:],
                                 start=True, stop=True)

            # ---- DRAM bounce tensors for the V exchange: 2 blocks of 4
            # k-tiles each (the AGs are latency-dominated, so fewer+bigger
            # beats many small ones), partition-major ([PT, 4*H] per rank
            # block) so the reload DMAs get 8KB per-partition descriptors.
            vh_d = [pdram.tile([PT, 4 * H], dt.bfloat16, tag=f"vhd{i}",
                               name="vhd") for i in range(2)]
            vf_d = [pdram.tile([2 * PT, 4 * H], dt.bfloat16, tag=f"vfd{i}",
                               name="vfd") for i in range(2)]

            # ---- dummy AllGather: fires immediately with no deps,
            # absorbing the first-collective setup/barrier latency so the
            # real V gathers start promptly once the CC stream is up. ----
            dum_i = pdram.tile([1, 64], dt.bfloat16, tag="dumi")
            dum_o = pdram.tile([2, 64], dt.bfloat16, tag="dumo")
            with tc.high_priority():
                nc.gpsimd.collective_compute(
                    "AllGather", mybir.AluOpType.bypass, replica_groups=GROUPS,
                    ins=[dum_i[:]], outs=[dum_o[:]],
                )

            # ---- input loads, batched into few multi-plane DMAs and
            # ordered by first use:
            #   scalar q: x8f plane-pairs (A1 rhs), then xq, wv (A2)
            #   sync   q: wk8 (A1 lhsT), biases; later the y stores
            #   gpsimd q: wq (A3); later vh exports / v peer reloads
            # x8f is chunk-major: [PT, 4 chunks, ND planes, 512] so phase A1
            # (chunk-outer) can start once the first chunk-pair lands.
            # Each load gets a distinct ascending priority: high_priority
            # blocks share priority 0 and interleave on the DMA ring
            # otherwise, which starves the phase-A1 loads (observed).
            # xq is chunk-major ([PT, 4, ND, 256]: 2 s-tiles per chunk) so
            # phase A2 starts once its first 0.5MB chunk lands; x8f likewise
            # for phase A1.
            x8f_t = pin.tile([PT, 4, ND, CH], dt.float8e4, tag="x8f")
            mt8_t = pin.tile([PT, ND, H], dt.float8e4, tag="mt8")
            xq_t = pin.tile([PT, 4, ND, QH // 4], dt.bfloat16, tag="xq")
            xq8_t = pin.tile([PT, ND, QH], dt.float8e4, tag="xq8")
            wv_t = pin.tile([PT, ND, H], dt.bfloat16, tag="wv")
            ut_t = pmisc.tile([PT, NKT], dt.float32, tag="ut")
            bv_t = pmisc.tile([PT, H], dt.bfloat16, tag="bv")

            def prio(i, fn):
                with tc.high_priority(offset=tc.cur_priority - i):
                    fn()

            prio(1, lambda: nc.sync.dma_start(out=mt8_t[:], in_=mt8[:, :]))
            for c in range(4):
                prio(2 + c, lambda c=c: nc.scalar.dma_start(
                    out=x8f_t[:, c, :, :],
                    in_=x8f[:, c * ND * CH:(c + 1) * ND * CH]))
            prio(6, lambda: nc.sync.dma_start(out=bv_t[:], in_=bvb[:, :]))
            prio(7, lambda: nc.sync.dma_start(out=wv_t[:], in_=wv[:, :]))
            for c in range(4):
                prio(8 + c, lambda c=c: nc.scalar.dma_start(
                    out=xq_t[:, c, :, :],
                    in_=xq[:, c * ND * (QH // 4):(c + 1) * ND * (QH // 4)]))
            prio(12, lambda: nc.scalar.dma_start(out=xq8_t[:], in_=xq8[:, :]))
            prio(13, lambda: nc.sync.dma_start(out=ut_t[:], in_=ut[:, :]))
            ones_t = pmisc.tile([PT, 1], dt.bfloat16, tag="ones")
            nc.vector.memset(ones_t[:], 1.0)

            # ---- phase A1: t^T = (32*Wq*Wk^T)^T . x^T over the full
            # sequence, fp8 DoubleRow, written straight into SBUF t8 (no
            # bias - bk cancels in softmax, bq folds into the exp bias).
            # Chunk-outer so compute starts as soon as the first x8f chunk
            # lands; d'-groups of 4 keep PSUM at 4 live banks. Exports
            # (plain fp8 casts) split between Vector and Scalar ACT so
            # neither engine gates the PE. ----
            t8 = pkt.tile([PT, ND, S], dt.float8e4, tag="t8")
            for c in range(4):
                for hh in range(2):
                    ps = [pp.tile([PT, CH], dt.float32, tag="big",
                                  name="psb") for _ in range(4)]
                    for dp in range(ND // 2):
                        rhs = x8f_t[:, c, 2 * dp:2 * dp + 2, :]
                        for h4 in range(4):
                            h = 4 * hh + h4
                            nc.tensor.matmul(
                                ps[h4][:],
                                mt8_t[:, 2 * dp:2 * dp + 2,
                                      h * PT:(h + 1) * PT],
                                rhs,
                                start=(dp == 0), stop=(dp == ND // 2 - 1),
                                perf_mode=DR,
                            )
                    for h4 in range(4):
                        h = 4 * hh + h4
                        dst = t8[:, h, c * CH:(c + 1) * CH]
                        if h4 % 2 == 0:
                            nc.vector.tensor_scalar_add(
                                dst, ps[h4][:], 0.0)
                        else:
                            nc.scalar.activation(dst, ps[h4][:], AF.Copy)

            # ---- phase A2: own-half V projection, st-major; exported to
            # DRAM for a pair AllGather per 4-tile block. Reloads go on
            # the sync queue so a pending reload never blocks the second
            # gather trigger on gpsimd. ----
            v_blk = {}
            for st in range(NST):
                ps = [pp.tile([PT, CH], dt.float32, tag="big", name="psb")
                      for _ in range(2)]
                for d in range(ND):
                    lhs = xq_t[:, st // 2, d,
                               (st % 2) * PT:(st % 2 + 1) * PT]
                    for hc in range(2):
                        nc.tensor.matmul(
                            ps[hc][:], lhs,
                            wv_t[:, d, hc * CH:(hc + 1) * CH],
                            start=(d == 0), stop=(d == ND - 1),
                        )
                half = pstage.tile([PT, H], dt.bfloat16, tag="halfv",
                                   name="halfv")
                with tc.high_priority():
                    for hc in range(2):
                        nc.vector.tensor_add(
                            half[:, hc * CH:(hc + 1) * CH], ps[hc][:],
                            bv_t[:, hc * CH:(hc + 1) * CH],
                        )
                    nc.gpsimd.dma_start(
                        out=vh_d[st // 4][:, (st % 4) * H:(st % 4 + 1) * H],
                        in_=half[:],
                    )
                if st % 4 == 3:
                    j = st // 4
                    with tc.high_priority():
                        nc.gpsimd.collective_compute(
                            "AllGather", mybir.AluOpType.bypass,
                            replica_groups=GROUPS,
                            ins=[vh_d[j][:]], outs=[vf_d[j][:]],
                        )
                        for r in range(2):
                            t = pv.tile([PT, 4 * H], dt.bfloat16,
                                        tag=f"vp{j}{r}", name="vp")
                            v_blk[j, r] = t
                            nc.sync.dma_start(
                                out=t[:],
                                in_=vf_d[j][r * PT:(r + 1) * PT, :])

            def v_tile(g):
                # global k-tile g -> SBUF [PT, H] view. vf_d[j] partition
                # block r, quarter i holds global k-tile r*8 + j*4 + i.
                return v_blk[(g % 8) // 4, g // 8][:, (g % 4) * H:
                                                  (g % 4) * H + H]

            # ---- phase B: scores^T + exp, fp8 DoubleRow: ST[k,q] =
            # t8[d,k]^T . xq8[d,q] (contraction over d). The exp folds the
            # 1/32 weight prescale into its scale and adds the per-k bq
            # term u_k as its per-partition bias. ET stored as 8 bf16
            # tiles [PT, 2*QH] (two k-tiles each). ----
            et_t = []
            for i in range(ND):
                et_t.append(px.tile([PT, 2 * QH], dt.bfloat16, tag="xt",
                                    name="et"))

            def et_slice(kt, q0, qn):
                return et_t[kt // 2][:, (kt % 2) * QH + q0:
                                     (kt % 2) * QH + q0 + qn]

            for kt in range(NKT):
                ps = [pp.tile([PT, CH], dt.float32, tag="big", name="psb")
                      for _ in range(2)]
                for dp in range(ND // 2):
                    lhs = t8[:, 2 * dp:2 * dp + 2, kt * PT:(kt + 1) * PT]
                    for qc in range(2):
                        nc.tensor.matmul(
                            ps[qc][:], lhs,
                            xq8_t[:, 2 * dp:2 * dp + 2,
                                  qc * CH:(qc + 1) * CH],
                            start=(dp == 0), stop=(dp == ND // 2 - 1),
                            perf_mode=DR,
                        )
                for qc in range(2):
                    nc.scalar.activation(
                        et_slice(kt, qc * CH, CH), ps[qc][:], AF.Exp,
                        scale=SCALE / MSC, bias=ut_t[:, kt:kt + 1],
                    )

            # ---- phase C: attn @ V, denominator, normalize. First-gathered
            # V blocks' k-tiles first, so a late AllGather cannot stall the
            # start of C. The last q-tile runs hc-split so its hc=0
            # normalize+store overlaps the hc=1 matmuls. ----
            KT_ORDER = [0, 1, 2, 3, 8, 9, 10, 11, 4, 5, 6, 7, 12, 13, 14, 15]
            for qt in range(NQT):
                dn = pp.tile([PT, 1], dt.float32, tag="big", name="dn")
                po = [pp.tile([PT, CH], dt.float32, tag="big", name="psb")
                      for _ in range(2)]
                rd = prd.tile([PT, 1], dt.float32, tag="rd")

                def emit_norm(hc):
                    stage = pstage.tile([PT, CH], dt.bfloat16, tag="st",
                                        name="stage")
                    nc.vector.tensor_scalar_mul(stage[:], po[hc][:], rd[:])
                    nc.sync.dma_start(
                        out=y[qt * PT:(qt + 1) * PT, hc * CH:(hc + 1) * CH],
                        in_=stage[:],
                    )

                if qt < NQT - 1:
                    for i, kt in enumerate(KT_ORDER):
                        lhs = et_slice(kt, qt * PT, PT)
                        nc.tensor.matmul(
                            dn[:], lhs, ones_t[:, 0:1],
                            start=(i == 0), stop=(i == NKT - 1),
                        )
                        for hc in range(2):
                            nc.tensor.matmul(
                                po[hc][:], lhs,
                                v_tile(kt)[:, hc * CH:(hc + 1) * CH],
                                start=(i == 0), stop=(i == NKT - 1),
                            )
                    nc.vector.reciprocal(rd[:], dn[:])
                    for hc in range(2):
                        emit_norm(hc)
                else:
                    for hc in range(2):
                        for i, kt in enumerate(KT_ORDER):
                            lhs = et_slice(kt, qt * PT, PT)
                            if hc == 0:
                                nc.tensor.matmul(
                                    dn[:], lhs, ones_t[:, 0:1],
                                    start=(i == 0), stop=(i == NKT - 1),
                                )
                            nc.tensor.matmul(
                                po[hc][:], lhs,
                                v_tile(kt)[:, hc * CH:(hc + 1) * CH],
                                start=(i == 0), stop=(i == NKT - 1),
                            )
                        if hc == 0:
                            nc.vector.reciprocal(rd[:], dn[:])
                            emit_norm(0)
                    emit_norm(1)

    return nc


def _get_nc():
    global _NC
    if _NC is None:
        nc = _build()
        nc.finalize()
        _NC = nc
    return _NC


def kernel(x, Wq, bq, Wk, bk, Wv, bv):
    from concourse.bass_utils import run_bass_kernel_spmd

    E4 = ml_dtypes.float8_e4m3

    def tl(a):
        # [D, X] -> SBUF tile layout [PT, ND*X]: tl[p, n*X + x] = a[n*PT+p, x]
        return np.ascontiguousarray(
            a.reshape(ND, PT, -1).transpose(1, 0, 2).reshape(PT, -1))

    # Fused score weights: scores = x (Wq Wk^T) x^T + (x Wk bq)_k; bk
    # cancels in the softmax. M is prescaled by MSC so its e4m3 encoding
    # stays in the normal range (entries ~1e-2 would land in subnormals).
    Mf = (Wq.astype(np.float32) @ Wk.astype(np.float32).T)
    mt_8 = tl((MSC * Mf.T).astype(BF16).astype(E4))
    wu = Wk.astype(np.float32) @ bq.astype(np.float32)
    wv_b = tl(Wv.astype(BF16))
    bv_b = np.ascontiguousarray(np.broadcast_to(bv.astype(BF16), (PT, H)))

    def tl_chunk(a, nch):
        # [D, X] -> chunk-major tile layout [PT, nch*ND*W], W = X/nch:
        # out[p, ((c*ND)+n)*W + k] = a[n*PT+p, c*W+k]
        w = a.shape[1] // nch
        return np.ascontiguousarray(
            a.reshape(ND, PT, nch, w).transpose(1, 2, 0, 3).reshape(PT, -1))

    xT_b = [x[b].T.astype(BF16) for b in range(B)]
    x8f_b = [tl_chunk(t.astype(E4), 4) for t in xT_b]
    xq_b = [[np.ascontiguousarray(t[:, qh * QH:(qh + 1) * QH])
             for qh in range(2)] for t in xT_b]
    ut_b = [np.ascontiguousarray(
        (x[b].astype(np.float32) @ wu * SCALE).reshape(NKT, PT).T)
        for b in range(B)]

    in_maps = []
    for c in range(NCORES):
        b, qh = divmod(c, 2)
        in_maps.append({
            "xq": tl_chunk(xq_b[b][qh], 4),
            "xq8": tl(xq_b[b][qh].astype(E4)),
            "x8f": x8f_b[b],
            "mt8": mt_8, "wv": wv_b,
            "ut": ut_b[b], "bvb": bv_b,
        })

    trace = bool(os.environ.get("BASS_KERNEL_TRACE"))
    kwargs = {}
    if trace:
        _register_ntff_hook()
        kwargs = {"trace": True,
                  "tmpdir": os.environ.get("BASS_KERNEL_TRACE_DIR")}

    nc = _get_nc()
    res = run_bass_kernel_spmd(nc, in_maps, list(range(NCORES)), **kwargs)
    if trace:
        kernel.last_exec_time_ns = res.exec_time_ns
        kernel.last_results = res

    out = np.empty((B, S, H), np.float32)
    for c in range(NCORES):
        b, qh = divmod(c, 2)
        out[b, qh * QH:(qh + 1) * QH, :] = np.asarray(
            res.results[c]["y"]).astype(np.float32)
    return out


def _register_ntff_hook():
    """The container's antenv lacks axon_hooks; register it so trace=True
    can capture NTFF profiles through the axon PJRT library."""
    import sys
    import types

    if "antenv.axon_hooks" in sys.modules:
        return
    mod = types.ModuleType("antenv.axon_hooks")
    holder = [None]
    mod.set_axon_ntff_profile_hook = lambda h: holder.__setitem__(0, h)
    mod.get_axon_ntff_profile_hook = lambda: holder[0]
    sys.modules["antenv.axon_hooks"] = mod
    import antenv

    antenv.axon_hooks = mod
    from trn_agent_boot.trn_boot import _ntff_profile_via_ctypes

    mod.set_axon_ntff_profile_hook(
        _ntff_profile_via_ctypes("/opt/axon/libaxon_pjrt.so"))
